# revision 33
# baseline (speedup 1.0000x reference)
"""GAT layer kernel for Trainium2, 8 NeuronCores.

Sharding: 16 (b, h) pairs -> 8 cores. Core k handles batch b = k//2 and the
head pair hp = k%2 (heads 2*hp, 2*hp+1); each core runs the full N^2
attention for its two heads. The head-mixing linear is split over nodes: the
pair of cores for one batch exchanges relu'd per-head outputs (cat halves)
with two small fp16 AllGathers (head 0's hides behind head 1's hot loop),
then each core computes the final linear + leaky for its own half of the
nodes only. No output AllReduce.

The program is identical on every core (SPMD): hp=1 cores get their node
axis block-swapped on the host (xT columns and madd rows+cols), so "own
half" is always local columns [0, N/2). Which AllGather slot holds the
peer's data is absorbed into the weights: the final linear contracts over
[own-cat, slot0-cat, slot1-cat] with the slot holding this core's own
sent-back data zeroed in wlT3.

Math per (b, h), with softmax over the *i* axis (rows) of e[i, j]:
  h    = x[b] @ W[h]                         [N, F]
  f1_i = h_i . a1,  f2_j = h_j . a2
  v[j, i]  = f1_i + f2_j + M[j, i]           (M = 0 on edge, -150 masked)
  L        = max(v, 0.2*v)                   (= leaky, exp-monotone safe)
  Em[j, i] = exp(L)    ;  s_j = sum_i Em[j, i]   (ACT accum_out, fused)
  g[j, :]  = h[j, :] / s_j
  hpT[f, i] = sum_j g[j, f] * Em[j, i]       (PE, transposed-out layout)
  out[own i] = leaky(relu(hp) cat-4-heads @ Wl.T + bl)

All PE inputs are fp16; the per-head projections are merged into single
[C, 2F] matmuls and f2 rides the same stationaries as h2. F1B (the broadcast
f1 row) comes from a host-replicated stationary (wab), so phase A needs no
ones-outer-product and no f1-row copies. The scalar engine (the pacer) runs
only: 4 F1B copies, then 32x (exp + accum) back to back.
"""

import sys

import numpy as np

sys.path.insert(0, "/opt/trn_rl_repo")

from concourse import bacc, bass, dve_ops, mybir, tile  # noqa: E402
from concourse.bass_utils import run_bass_kernel_spmd  # noqa: E402
from concourse.dve_spec import C0, C1, C2, Spec, Src0, Src1, relu  # noqa: E402

# Fused leaky-relu of a masked outer sum, one DVE pass at 1x:
#   out = leaky(in0 + s0 + in1) = s1*v + imm2*relu(v),  v = in0 + s0 + in1
# (in0 = broadcast f1 row, s0 = per-partition f2, in1 = additive adj mask).
_v = (Src0 + C0) + Src1
LEAKY_MASK_ANT = dve_ops.DveOp(
    "LEAKY_MASK_ANT",
    Spec(
        body=_v * C1 + relu(_v) * C2,
        reference=lambda in0, in1, s0, s1, imm2: (
            lambda v: (v * s1 + np.maximum(v, 0) * imm2).astype(np.float32)
        )(in0.astype(np.float32) + s0 + in1),
    ),
    subdim=False,
    uops_sha={"v3": "61445124be53cf8e", "v4": "fd84e7f03d2c00e0"},
)
if LEAKY_MASK_ANT.name not in dve_ops._SUB_OPCODE_FOR_NAME:
    dve_ops.OPS.append(LEAKY_MASK_ANT)
    dve_ops._SUB_OPCODE_FOR_NAME[LEAKY_MASK_ANT.name] = (
        dve_ops._CUSTOM_DVE_ROW_BASE + len(dve_ops.OPS) - 1)
    dve_ops.CUSTOM_DVE_SPECS[LEAKY_MASK_ANT.name] = LEAKY_MASK_ANT.spec

# Mask-after-leaky variant with a hand-authored 2x_1PORT uop program:
#   out = max(a, a*s1) + in1,  a = in0 + s0
# (identical math given the additive mask rides outside the leaky: exp of
# both is ~0 at masked entries). 4 ALU stages -> lo element in slices 0-3,
# hi element (SRC_*_HI) in slices 4-7, lo result rides the delay chain out
# through WR0_LO=DELAY_0 while the hi result exits via ALU_OUT.
from concourse.dve_spec import lower as _dve_lower, maxx  # noqa: E402
from concourse.dve_uop import (  # noqa: E402
    AluInp, AluOp as UAluOp, DelayInp, DveOpSpec, InpSel, OutPath, OutSel,
)


def _leaky_pm_2x_uop(u1x):
    import copy as _copy
    u = _copy.deepcopy(u1x)
    u.inp = [InpSel.ZERO, InpSel.SRC_0, InpSel.CONST_0, InpSel.CONST_1,
             InpSel.SRC_1, InpSel.SRC_0_HI, InpSel.SRC_1_HI, InpSel.ZERO]
    u.inp_enable = [0, 1, 1, 1, 1, 1, 1, 0]
    u.out = {OutPath.WR0_LO: OutSel.DELAY_0, OutPath.WR0_HI: OutSel.ALU_OUT,
             OutPath.WR1_LO: OutSel.ALU_OUT, OutPath.WR1_HI: OutSel.ALU_OUT}
    u.out_enable = {OutPath.WR0_LO: 1, OutPath.WR0_HI: 1,
                    OutPath.WR1_LO: 0, OutPath.WR1_HI: 0}

    def mk(ref, op, a, b, cap=None):
        dp = _copy.deepcopy(ref)
        dp.op = op
        dp.alu_src0 = a
        dp.alu_src1 = b
        dp.delay = [DelayInp.PREV_DELAY] * 6 + [DelayInp.PREV_ALU_OUT]
        dp.delay_enable = [1, 1, 1, 1, 1, 1, 0]
        dp.alu_out_enable = 1
        if cap is not None:
            dp.delay[cap] = DelayInp.PREV_ALU_OUT
        return dp

    r = u1x.datapath_config[0]
    PA, PD = AluInp.PREV_ALU_OUT, [
        AluInp.PREV_DELAY_0, AluInp.PREV_DELAY_1, AluInp.PREV_DELAY_2,
        AluInp.PREV_DELAY_3, AluInp.PREV_DELAY_4, AluInp.PREV_DELAY_5]
    u.datapath_config = [
        mk(r, UAluOp.ADD, PD[0], PD[1]),            # a_lo = Src0 + C0
        mk(r, UAluOp.MULTIPLY, PA, PD[2], cap=0),   # a_lo*s1; PD0 := a_lo
        mk(r, UAluOp.MAX, PD[0], PA),               # max(a_lo, a_lo*s1)
        mk(r, UAluOp.ADD, PA, PD[3]),               # + Src1 -> lo result
        mk(r, UAluOp.ADD, PD[4], PD[1], cap=0),     # a_hi; PD0 := lo result
        mk(r, UAluOp.MULTIPLY, PA, PD[2], cap=4),   # a_hi*s1; PD4 := a_hi
        mk(r, UAluOp.MAX, PD[4], PA),               # max(a_hi, a_hi*s1)
        mk(r, UAluOp.ADD, PA, PD[5]),               # + Src1_HI -> hi result
    ]
    return u


class _DveOpPerf(dve_ops.DveOp):
    """DveOp whose compiled spec carries a hand-authored 2x_1PORT program."""

    def compile(self, ver):
        key = (self.name, ver)
        if (r := dve_ops._COMPILE_CACHE.get(key)) is not None:
            return r
        uops = _dve_lower(self.spec, ver=ver)
        result = DveOpSpec(
            name=self.name,
            opcode=dve_ops.get_dve_sub_opcode(self.name),
            uops=uops,
            uops_2x=[_leaky_pm_2x_uop(uops[0])],
            perf_max=1,
            rd1_en=True,
        )
        dve_ops._COMPILE_CACHE[key] = result
        return result


_a = Src0 + C0
LEAKY_PM2X_ANT = _DveOpPerf(
    "LEAKY_PM2X_ANT",
    Spec(
        body=maxx(_a, _a * C1) + Src1,
        reference=lambda in0, in1, s0, s1, imm2=0.0: (
            lambda a: (np.maximum(a, a * s1) + in1).astype(np.float32)
        )(in0.astype(np.float32) + s0),
    ),
    subdim=False,
    uops_sha={},
)
if LEAKY_PM2X_ANT.name not in dve_ops._SUB_OPCODE_FOR_NAME:
    dve_ops.OPS.append(LEAKY_PM2X_ANT)
    dve_ops._SUB_OPCODE_FOR_NAME[LEAKY_PM2X_ANT.name] = (
        dve_ops._CUSTOM_DVE_ROW_BASE + len(dve_ops.OPS) - 1)
    dve_ops.CUSTOM_DVE_SPECS[LEAKY_PM2X_ANT.name] = LEAKY_PM2X_ANT.spec

B, N, C, F, H = 4, 2048, 256, 64, 4
P = 128
NT = N // P  # 16 j-tiles / n-chunks
CT = C // P  # 2 contraction tiles over Cin
IC = 512  # i-chunk (matmul moving free dim / psum bank)
NIC = N // IC  # 4
F2 = 2 * F  # merged two-head feature dim (128)
NH = N // 2  # per-core output node half (1024)
NCH = NH // P  # 8 output row chunks
ALPHA = 0.2
MASKV = 150.0  # additive mask magnitude; exp(0.2 * -150) ~ 1e-13
NCORES = 8

F32 = mybir.dt.float32
F16 = mybir.dt.float16
ADD = mybir.AluOpType.add
MULT = mybir.AluOpType.mult
MAX = mybir.AluOpType.max
BYPASS = mybir.AluOpType.bypass
EXPF = mybir.ActivationFunctionType.Exp
PAIRS = [[0, 1], [2, 3], [4, 5], [6, 7]]

_CACHE = {}


def _build_program():
    nc = bacc.Bacc("TRN2", target_bir_lowering=False, debug=False,
                   num_devices=NCORES)

    xT = nc.dram_tensor("xT", [C, N], F16, kind="ExternalInput")
    madd = nc.dram_tensor("madd", [N, N], F16, kind="ExternalInput")
    # w2a[:, 0:128] = per-head W (both local heads); [:, 128:130] = W@a2 cols
    w2a = nc.dram_tensor("w2a", [C, F2 + 2], F16, kind="ExternalInput")
    # wab[:, hl, :] = (W[h] @ a1[h]) column broadcast to 128 -> F1B stationary
    wab = nc.dram_tensor("wab", [C, 2, P], F16, kind="ExternalInput")
    # wlT3[0] = own heads' Wl.T rows; wlT3[1]/[2] = AllGather slot-0/1 heads'
    # rows, with the slot holding this core's own sent-back data zeroed.
    wlT3 = nc.dram_tensor("wlT3", [3, P, F], F16, kind="ExternalInput")
    blc = nc.dram_tensor("blc", [F, 1], F32, kind="ExternalInput")
    # transposed output [F, NH]; the host transposes while gathering
    out = nc.dram_tensor("out", [F, NH], F32, kind="ExternalOutput")

    cc_in = [nc.dram_tensor(f"cc_in{hl}", [F, NH], F16) for hl in range(2)]
    cc_out = [nc.dram_tensor(f"cc_out{hl}", [2, F, NH], F16)
              for hl in range(2)]

    with tile.TileContext(nc) as tc:
        with (
            tc.tile_pool(name="const", bufs=1) as const,
            tc.tile_pool(name="vm", bufs=3) as vm_pool,
            tc.tile_pool(name="em", bufs=3) as em_pool,
            tc.tile_pool(name="g", bufs=4) as g_pool,
            tc.tile_pool(name="psA", bufs=2, space="PSUM") as psA,
            tc.tile_pool(name="psB", bufs=1, space="PSUM") as psB,
            tc.tile_pool(name="psX", bufs=2, space="PSUM") as psX,
        ):
            # ---- DMA issue: xT quarters split over all three DMA queues
            # (after each queue's small params) so the last piece lands
            # ~4us earlier than a single sync stream; madd split
            # across sync+gpsimd -------------------------------------------
            xT_sb = const.tile([P, CT, N], F16)

            def xt_piece(eng, ct, nh):
                eng.dma_start(
                    xT_sb[:, ct, nh * NH:(nh + 1) * NH],
                    xT[ct * P:(ct + 1) * P, nh * NH:(nh + 1) * NH])

            xt_piece(nc.sync, 0, 0)
            wab_sb = const.tile([P, CT, 2, P], F16)
            for ct in range(CT):
                nc.scalar.dma_start(wab_sb[:, ct], wab[ct * P:(ct + 1) * P])
            wlT3_sb = const.tile([P, 3, F], F16)
            for c in range(3):
                nc.gpsimd.dma_start(wlT3_sb[:, c], wlT3[c])
            blc_sb = const.tile([F, 1], F32)
            nc.gpsimd.dma_start(blc_sb[:], blc[:])
            xt_piece(nc.scalar, 1, 0)   # scalar: wab then this piece
            xt_piece(nc.gpsimd, 0, 1)   # gpsimd: wlT3/blc then this piece
            xt_piece(nc.sync, 1, 1)
            w2a_sb = const.tile([P, CT, F2 + 2], F16)
            for ct in range(CT):
                nc.scalar.dma_start(w2a_sb[:, ct, :],
                                    w2a[ct * P:(ct + 1) * P, :])
            madd_sb = [const.tile([P, N], F16, tag=f"madd{j}",
                                  name=f"madd_sb{j}")
                       for j in range(NT)]
            for jt in range(NT):
                eng = nc.sync if jt % 2 == 0 else nc.gpsimd
                eng.dma_start(madd_sb[jt][:], madd[jt * P:(jt + 1) * P, :])

            # ---- phase A -------------------------------------------------
            F1B_sb = [const.tile([P, N], F16, tag=f"f1b{hl}",
                                 name=f"F1B_sb{hl}") for hl in range(2)]
            h2_sb = const.tile([P, NT, F2], F16)  # [n, nt, 2F] both heads
            f2c_sb = const.tile([P, 2, NT], F32)
            sc_sb = const.tile([P, 2, NT], F32)
            rc_sb = const.tile([P, 2, NT], F32)

            # F1B head 0 via the host-replicated stationary; copies on the
            # scalar engine (they precede all exps in its FIFO). F1B psums
            # live in psX so the h2 stream (psA) doesn't gate their rotation.
            # high_priority keeps the scheduler from statically interleaving
            # the h2 matmul stream ahead of this EXP[0]-critical chain.
            with tc.high_priority():
                for icc in range(NIC):
                    ps = psX.tile([P, IC], F32, tag="psum_x")
                    for ct in range(CT):
                        nc.tensor.matmul(
                            ps[:], wab_sb[:, ct, 0],
                            xT_sb[:, ct, icc * IC:(icc + 1) * IC],
                            start=(ct == 0), stop=(ct == CT - 1))
                    nc.scalar.copy(F1B_sb[0][:, icc * IC:(icc + 1) * IC],
                                   ps[:])

            # h2 + f2 share stationaries (xT chunks); f2 in its own psum,
            # one tile per 4-nt group so grp0's f2 column (the EXP[0] gate)
            # is copyable right after grp0's matmuls. h2/f2 copies for
            # grp >= 1 are deferred into loop slack.
            f2ps = [psX.tile([P, 2, 4], F32, tag="psum_x", name=f"f2ps{i}")
                    for i in range(4)]
            h2grp = []
            for grp in range(4):
                ps = psA.tile([P, 4, F2], F32, tag="psum_a")
                h2grp.append(ps)
                for k in range(4):
                    nt = grp * 4 + k
                    for ct in range(CT):
                        nc.tensor.matmul(
                            ps[:, k, :],
                            xT_sb[:, ct, nt * P:(nt + 1) * P],
                            w2a_sb[:, ct, :F2],
                            start=(ct == 0), stop=(ct == CT - 1))
                        nc.tensor.matmul(
                            f2ps[grp][:, :, k],
                            xT_sb[:, ct, nt * P:(nt + 1) * P],
                            w2a_sb[:, ct, F2:],
                            start=(ct == 0), stop=(ct == CT - 1))
            nc.vector.tensor_copy(h2_sb[:, 0:4, :], h2grp[0][:])
            nc.vector.tensor_copy(f2c_sb[:, :, 0:4], f2ps[0][:])

            # ---- hot loop (+ deferred off-critical work in loop slack) ---
            hpT2 = psB.tile([P, N], F32, tag="hpT")
            catT_own = const.tile([P, NH], F16)
            catS = [const.tile([P, NH], F16, tag=f"catS{s}",
                               name=f"catS{s}") for s in range(2)]
            send_sb = const.tile([P, NH], F16)

            def deferred(hl, jt):
                if hl == 0 and 1 <= jt <= 3:
                    # h2 + f2 psum copies for grp 1..3 (deferred off the
                    # EXP[0]-critical vector queue prefix)
                    grp = jt
                    nc.vector.tensor_copy(
                        f2c_sb[:, :, grp * 4:(grp + 1) * 4], f2ps[grp][:])
                    nc.vector.tensor_copy(
                        h2_sb[:, grp * 4:(grp + 1) * 4, :], h2grp[grp][:])
                if hl == 0 and 6 <= jt < 6 + NIC:
                    # F1B head 1, one i-chunk per slot (vector copies)
                    icc = jt - 6
                    ps = psX.tile([P, IC], F32, tag="psum_x")
                    for ct in range(CT):
                        nc.tensor.matmul(
                            ps[:], wab_sb[:, ct, 1],
                            xT_sb[:, ct, icc * IC:(icc + 1) * IC],
                            start=(ct == 0), stop=(ct == CT - 1))
                    nc.vector.tensor_copy(
                        F1B_sb[1][:, icc * IC:(icc + 1) * IC], ps[:])
                if hl == 1 and jt == 7:
                    nc.sync.dma_start(catS[0][0:F, :], cc_out[0][0])
                    nc.sync.dma_start(catS[1][0:F, :], cc_out[0][1])

            for hl in range(2):
                prev = None  # software pipeline: custom(jt+1) before norm(jt)
                for jt in range(NT):
                    lk = vm_pool.tile([P, N], F16, tag="lk")
                    bi = nc.vector._custom_dve(
                        LEAKY_PM2X_ANT, out=lk[:], in0=F1B_sb[hl][:],
                        in1=madd_sb[jt][:], s0=f2c_sb[:, hl, jt:jt + 1],
                        s1=float(ALPHA))
                    bi.ins.perf_max = 1
                    em = em_pool.tile([P, N], F16, tag="em")
                    nc.scalar.activation(em[:], lk[:], EXPF,
                                         accum_out=sc_sb[:, hl, jt:jt + 1])
                    deferred(hl, jt)
                    if prev is not None:
                        _emit_norm_mm(nc, prev, hl, h2_sb, sc_sb, rc_sb,
                                      g_pool, hpT2)
                    prev = (jt, em)
                # flush: send-half (upper) i-chunks first so their psum
                # banks close and the exchange relu can fire 2 MMs early
                g = _emit_norm_mm(nc, prev, hl, h2_sb, sc_sb, rc_sb,
                                  g_pool, hpT2, iccs=(2, 3))
                nc.vector.tensor_scalar(
                    send_sb[hl * F:(hl + 1) * F, :],
                    hpT2[hl * F:(hl + 1) * F, NH:], 0.0, None, op0=MAX)
                nc.sync.dma_start(cc_in[hl][:],
                                  send_sb[hl * F:(hl + 1) * F, :])
                nc.gpsimd.collective_compute(
                    "AllGather", BYPASS, replica_groups=PAIRS,
                    ins=[cc_in[hl][:]], outs=[cc_out[hl][:]])
                _emit_norm_mm(nc, prev, hl, h2_sb, sc_sb, rc_sb,
                              g_pool, hpT2, iccs=(0, 1), g=g)

            # ---- head-1 epilogue: own-half relus on the idle scalar ------
            nc.scalar.activation(catT_own[0:F, :], hpT2[0:F, :NH],
                                 mybir.ActivationFunctionType.Relu)
            nc.scalar.activation(catT_own[F:F2, :], hpT2[F:F2, :NH],
                                 mybir.ActivationFunctionType.Relu)

            # ---- phase C: final linear over own node half, transposed ----
            # out[f, i] = sum_hf wlT[hf, f] cat[hf, i]: stationary = wl
            # chunks (one LDW each), moving = cat tiles. Own-cat matmuls
            # overlap the gather; two psum banks (i halves).
            psD = [psX.tile([F, IC], F32, tag="psum_x", name=f"psD{i}")
                   for i in range(2)]
            for ih in range(2):
                nc.tensor.matmul(
                    psD[ih][:], wlT3_sb[:, 0],
                    catT_own[:, ih * IC:(ih + 1) * IC],
                    start=True, stop=False)
            nc.sync.dma_start(catS[0][F:F2, :], cc_out[1][0])
            nc.sync.dma_start(catS[1][F:F2, :], cc_out[1][1])
            for ih in range(2):
                for s in range(2):
                    nc.tensor.matmul(
                        psD[ih][:], wlT3_sb[:, 1 + s],
                        catS[s][:, ih * IC:(ih + 1) * IC],
                        start=False, stop=(s == 1))
            # bias (per-partition, fused into the psum drain on scalar),
            # then leaky on vector; out stays [F, NH] (host transposes)
            y_sb = const.tile([F, 2, IC], F32)
            for ih in range(2):
                nc.scalar.activation(
                    y_sb[:, ih, :], psD[ih][:],
                    mybir.ActivationFunctionType.Identity, bias=blc_sb[:])
            yo_sb = const.tile([F, 2, IC], F32)
            nc.vector.scalar_tensor_tensor(
                yo_sb[:], y_sb[:], float(ALPHA), y_sb[:], op0=MULT, op1=MAX)
            nc.sync.dma_start(out.rearrange("f (h i) -> f h i", h=2),
                              yo_sb[:])

    nc.compile()
    return nc


def _emit_norm_mm(nc, prev, hl, h2_sb, sc_sb, rc_sb, g_pool, hpT2,
                  iccs=None, g=None):
    """Normalization + attention matmuls for a finished (jt, em) stage."""
    jt, em = prev
    if g is None:
        nc.vector.reciprocal(rc_sb[:, hl, jt:jt + 1],
                             sc_sb[:, hl, jt:jt + 1])
        g = g_pool.tile([P, F], F16, tag="g")
        nc.vector.tensor_scalar_mul(g[:], h2_sb[:, jt, hl * F:(hl + 1) * F],
                                    rc_sb[:, hl, jt:jt + 1])
    for icc in (range(NIC) if iccs is None else iccs):
        nc.tensor.matmul(
            hpT2[hl * F:(hl + 1) * F, icc * IC:(icc + 1) * IC],
            g[:], em[:, icc * IC:(icc + 1) * IC],
            start=(jt == 0), stop=(jt == NT - 1))
    return g


def get_program():
    if "nc" not in _CACHE:
        _CACHE["nc"] = _build_program()
    return _CACHE["nc"]


def make_in_maps(x, adj, W, a1, a2, Wl, bl):
    x = np.asarray(x, dtype=np.float32)
    adj = np.asarray(adj)
    W = np.asarray(W, dtype=np.float32)
    a1 = np.asarray(a1, dtype=np.float32)
    a2 = np.asarray(a2, dtype=np.float32)
    Wl = np.asarray(Wl, dtype=np.float32)
    bl = np.asarray(bl, dtype=np.float32)

    maddT = (MASKV * adj.T.astype(np.float32)) - MASKV  # [j, i] additive
    madds = [
        np.ascontiguousarray(maddT).astype(np.float16),
        # hp=1: node axis block-swapped on both j and i
        np.ascontiguousarray(
            np.roll(np.roll(maddT, NH, axis=0), NH, axis=1)
        ).astype(np.float16),
    ]
    WlT = np.ascontiguousarray(Wl.T)  # [H*F, F]
    zero_blk = np.zeros((P, F), np.float32)

    in_maps = []
    for k in range(NCORES):
        b, hp = k // 2, k % 2
        w2a = np.concatenate([
            W[2 * hp], W[2 * hp + 1],
            (W[2 * hp] @ a2[2 * hp])[:, None],
            (W[2 * hp + 1] @ a2[2 * hp + 1])[:, None],
        ], axis=1).astype(np.float16)  # [C, 130]
        wab = np.stack([
            np.tile((W[2 * hp] @ a1[2 * hp])[:, None], (1, P)),
            np.tile((W[2 * hp + 1] @ a1[2 * hp + 1])[:, None], (1, P)),
        ], axis=1).astype(np.float16)  # [C, 2, P]
        own = WlT[hp * P:(hp + 1) * P]
        slot0 = zero_blk if hp == 0 else WlT[0:P]       # rank-0 heads (0, 1)
        slot1 = WlT[P:2 * P] if hp == 0 else zero_blk   # rank-1 heads (2, 3)
        wlT3 = np.stack([own, slot0, slot1], axis=0).astype(np.float16)
        xb = x[b].T  # [C, N]
        if hp == 1:
            xb = np.roll(xb, NH, axis=1)
        in_maps.append({
            "xT": np.ascontiguousarray(xb).astype(np.float16),
            "madd": madds[hp],
            "w2a": np.ascontiguousarray(w2a),
            "wab": np.ascontiguousarray(wab),
            "wlT3": np.ascontiguousarray(wlT3),
            "blc": np.ascontiguousarray(bl[:, None]).astype(np.float32),
        })
    return in_maps


def kernel(x, adj, W, a1, a2, Wl, bl, _results=None, **run_kwargs):
    nc = get_program()
    in_maps = make_in_maps(x, adj, W, a1, a2, Wl, bl)
    res = run_bass_kernel_spmd(nc, in_maps, core_ids=list(range(NCORES)),
                               **run_kwargs)
    if _results is not None:
        _results.append(res)
    out = np.empty((B, N, F), dtype=np.float32)
    for b in range(B):
        out[b, :NH] = res.results[2 * b]["out"].T
        out[b, NH:] = res.results[2 * b + 1]["out"].T
    return out


# revision 36
# speedup vs baseline: 1.0652x; 1.0652x over previous
"""GAT layer kernel for Trainium2, 8 NeuronCores.

Sharding: 16 (b, h) pairs -> 8 cores. Core k handles batch b = k//2 and the
head pair hp = k%2 (heads 2*hp, 2*hp+1); each core runs the full N^2
attention for its two heads. The head-mixing linear is split over nodes: the
pair of cores for one batch exchanges relu'd per-head outputs (cat halves)
with two small fp16 AllGathers (head 0's hides behind head 1's hot loop),
then each core computes the final linear + leaky for its own half of the
nodes only. No output AllReduce.

The program is identical on every core (SPMD): hp=1 cores get their node
axis block-swapped on the host (xT columns and madd rows+cols), so "own
half" is always local columns [0, N/2). Which AllGather slot holds the
peer's data is absorbed into the weights: the final linear contracts over
[own-cat, slot0-cat, slot1-cat] with the slot holding this core's own
sent-back data zeroed in wlT3.

Math per (b, h), with softmax over the *i* axis (rows) of e[i, j]:
  h    = x[b] @ W[h]                         [N, F]
  f1_i = h_i . a1,  f2_j = h_j . a2
  v[j, i]  = f1_i + f2_j + M[j, i]           (M = 0 on edge, -150 masked)
  L        = max(v, 0.2*v)                   (= leaky, exp-monotone safe)
  Em[j, i] = exp(L)    ;  s_j = sum_i Em[j, i]   (ACT accum_out, fused)
  g[j, :]  = h[j, :] / s_j
  hpT[f, i] = sum_j g[j, f] * Em[j, i]       (PE, transposed-out layout)
  out[own i] = leaky(relu(hp) cat-4-heads @ Wl.T + bl)

All PE inputs are fp16; the per-head projections are merged into single
[C, 2F] matmuls and f2 rides the same stationaries as h2. F1B (the broadcast
f1 row) comes from a host-replicated stationary (wab), so phase A needs no
ones-outer-product and no f1-row copies. The scalar engine (the pacer) runs
only: 4 F1B copies, then 32x (exp + accum) back to back.
"""

import sys

import numpy as np

sys.path.insert(0, "/opt/trn_rl_repo")

from concourse import bacc, bass, dve_ops, mybir, tile  # noqa: E402
from concourse.bass_utils import run_bass_kernel_spmd  # noqa: E402
from concourse.dve_spec import C0, C1, C2, Spec, Src0, Src1, relu  # noqa: E402

# Fused leaky-relu of a masked outer sum, one DVE pass at 1x:
#   out = leaky(in0 + s0 + in1) = s1*v + imm2*relu(v),  v = in0 + s0 + in1
# (in0 = broadcast f1 row, s0 = per-partition f2, in1 = additive adj mask).
_v = (Src0 + C0) + Src1
LEAKY_MASK_ANT = dve_ops.DveOp(
    "LEAKY_MASK_ANT",
    Spec(
        body=_v * C1 + relu(_v) * C2,
        reference=lambda in0, in1, s0, s1, imm2: (
            lambda v: (v * s1 + np.maximum(v, 0) * imm2).astype(np.float32)
        )(in0.astype(np.float32) + s0 + in1),
    ),
    subdim=False,
    uops_sha={"v3": "61445124be53cf8e", "v4": "fd84e7f03d2c00e0"},
)
if LEAKY_MASK_ANT.name not in dve_ops._SUB_OPCODE_FOR_NAME:
    dve_ops.OPS.append(LEAKY_MASK_ANT)
    dve_ops._SUB_OPCODE_FOR_NAME[LEAKY_MASK_ANT.name] = (
        dve_ops._CUSTOM_DVE_ROW_BASE + len(dve_ops.OPS) - 1)
    dve_ops.CUSTOM_DVE_SPECS[LEAKY_MASK_ANT.name] = LEAKY_MASK_ANT.spec

# Mask-after-leaky variant with a hand-authored 2x_1PORT uop program:
#   out = max(a, a*s1) + in1,  a = in0 + s0
# (identical math given the additive mask rides outside the leaky: exp of
# both is ~0 at masked entries). 4 ALU stages -> lo element in slices 0-3,
# hi element (SRC_*_HI) in slices 4-7, lo result rides the delay chain out
# through WR0_LO=DELAY_0 while the hi result exits via ALU_OUT.
from concourse.dve_spec import lower as _dve_lower, maxx  # noqa: E402
from concourse.dve_uop import (  # noqa: E402
    AluInp, AluOp as UAluOp, DelayInp, DveOpSpec, InpSel, OutPath, OutSel,
)


def _leaky_pm_2x_uop(u1x):
    import copy as _copy
    u = _copy.deepcopy(u1x)
    u.inp = [InpSel.ZERO, InpSel.SRC_0, InpSel.CONST_0, InpSel.CONST_1,
             InpSel.SRC_1, InpSel.SRC_0_HI, InpSel.SRC_1_HI, InpSel.ZERO]
    u.inp_enable = [0, 1, 1, 1, 1, 1, 1, 0]
    u.out = {OutPath.WR0_LO: OutSel.DELAY_0, OutPath.WR0_HI: OutSel.ALU_OUT,
             OutPath.WR1_LO: OutSel.ALU_OUT, OutPath.WR1_HI: OutSel.ALU_OUT}
    u.out_enable = {OutPath.WR0_LO: 1, OutPath.WR0_HI: 1,
                    OutPath.WR1_LO: 0, OutPath.WR1_HI: 0}

    def mk(ref, op, a, b, cap=None):
        dp = _copy.deepcopy(ref)
        dp.op = op
        dp.alu_src0 = a
        dp.alu_src1 = b
        dp.delay = [DelayInp.PREV_DELAY] * 6 + [DelayInp.PREV_ALU_OUT]
        dp.delay_enable = [1, 1, 1, 1, 1, 1, 0]
        dp.alu_out_enable = 1
        if cap is not None:
            dp.delay[cap] = DelayInp.PREV_ALU_OUT
        return dp

    r = u1x.datapath_config[0]
    PA, PD = AluInp.PREV_ALU_OUT, [
        AluInp.PREV_DELAY_0, AluInp.PREV_DELAY_1, AluInp.PREV_DELAY_2,
        AluInp.PREV_DELAY_3, AluInp.PREV_DELAY_4, AluInp.PREV_DELAY_5]
    u.datapath_config = [
        mk(r, UAluOp.ADD, PD[0], PD[1]),            # a_lo = Src0 + C0
        mk(r, UAluOp.MULTIPLY, PA, PD[2], cap=0),   # a_lo*s1; PD0 := a_lo
        mk(r, UAluOp.MAX, PD[0], PA),               # max(a_lo, a_lo*s1)
        mk(r, UAluOp.ADD, PA, PD[3]),               # + Src1 -> lo result
        mk(r, UAluOp.ADD, PD[4], PD[1], cap=0),     # a_hi; PD0 := lo result
        mk(r, UAluOp.MULTIPLY, PA, PD[2], cap=4),   # a_hi*s1; PD4 := a_hi
        mk(r, UAluOp.MAX, PD[4], PA),               # max(a_hi, a_hi*s1)
        mk(r, UAluOp.ADD, PA, PD[5]),               # + Src1_HI -> hi result
    ]
    return u


class _DveOpPerf(dve_ops.DveOp):
    """DveOp whose compiled spec carries a hand-authored 2x_1PORT program."""

    def compile(self, ver):
        key = (self.name, ver)
        if (r := dve_ops._COMPILE_CACHE.get(key)) is not None:
            return r
        uops = _dve_lower(self.spec, ver=ver)
        result = DveOpSpec(
            name=self.name,
            opcode=dve_ops.get_dve_sub_opcode(self.name),
            uops=uops,
            uops_2x=[_leaky_pm_2x_uop(uops[0])],
            perf_max=1,
            rd1_en=True,
        )
        dve_ops._COMPILE_CACHE[key] = result
        return result


_a = Src0 + C0
LEAKY_PM2X_ANT = _DveOpPerf(
    "LEAKY_PM2X_ANT",
    Spec(
        body=maxx(_a, _a * C1) + Src1,
        reference=lambda in0, in1, s0, s1, imm2=0.0: (
            lambda a: (np.maximum(a, a * s1) + in1).astype(np.float32)
        )(in0.astype(np.float32) + s0),
    ),
    subdim=False,
    uops_sha={},
)
if LEAKY_PM2X_ANT.name not in dve_ops._SUB_OPCODE_FOR_NAME:
    dve_ops.OPS.append(LEAKY_PM2X_ANT)
    dve_ops._SUB_OPCODE_FOR_NAME[LEAKY_PM2X_ANT.name] = (
        dve_ops._CUSTOM_DVE_ROW_BASE + len(dve_ops.OPS) - 1)
    dve_ops.CUSTOM_DVE_SPECS[LEAKY_PM2X_ANT.name] = LEAKY_PM2X_ANT.spec

B, N, C, F, H = 4, 2048, 256, 64, 4
P = 128
NT = N // P  # 16 j-tiles / n-chunks
CT = C // P  # 2 contraction tiles over Cin
IC = 512  # i-chunk (matmul moving free dim / psum bank)
NIC = N // IC  # 4
F2 = 2 * F  # merged two-head feature dim (128)
NH = N // 2  # per-core output node half (1024)
NCH = NH // P  # 8 output row chunks
ALPHA = 0.2
MASKV = 150.0  # additive mask magnitude; exp(0.2 * -150) ~ 1e-13
NCORES = 8

F32 = mybir.dt.float32
F16 = mybir.dt.float16
ADD = mybir.AluOpType.add
MULT = mybir.AluOpType.mult
MAX = mybir.AluOpType.max
BYPASS = mybir.AluOpType.bypass
EXPF = mybir.ActivationFunctionType.Exp
PAIRS = [[0, 1], [2, 3], [4, 5], [6, 7]]

_CACHE = {}


def _build_program():
    nc = bacc.Bacc("TRN2", target_bir_lowering=False, debug=False,
                   num_devices=NCORES)

    xT = nc.dram_tensor("xT", [C, N], F16, kind="ExternalInput")
    madd = nc.dram_tensor("madd", [N, N], F16, kind="ExternalInput")
    # w2a[:, 0:128] = per-head W (both local heads); [:, 128:130] = W@a2 cols
    w2a = nc.dram_tensor("w2a", [C, F2 + 2], F16, kind="ExternalInput")
    # wab[:, hl, :] = (W[h] @ a1[h]) column broadcast to 128 -> F1B stationary
    wab = nc.dram_tensor("wab", [C, 2, P], F16, kind="ExternalInput")
    # wlT3[0] = own heads' Wl.T rows; wlT3[1]/[2] = AllGather slot-0/1 heads'
    # rows, with the slot holding this core's own sent-back data zeroed.
    wlT3 = nc.dram_tensor("wlT3", [3, P, F], F16, kind="ExternalInput")
    blc = nc.dram_tensor("blc", [F, 1], F32, kind="ExternalInput")
    # transposed output [F, NH]; the host transposes while gathering
    out = nc.dram_tensor("out", [F, NH], F32, kind="ExternalOutput")

    cc_in = [nc.dram_tensor(f"cc_in{hl}", [F, NH], F16) for hl in range(2)]
    cc_out = [nc.dram_tensor(f"cc_out{hl}", [2, F, NH], F16)
              for hl in range(2)]

    with tile.TileContext(nc) as tc:
        with (
            tc.tile_pool(name="const", bufs=1) as const,
            tc.tile_pool(name="vm", bufs=3) as vm_pool,
            tc.tile_pool(name="em", bufs=3) as em_pool,
            tc.tile_pool(name="g", bufs=4) as g_pool,
            tc.tile_pool(name="psA", bufs=2, space="PSUM") as psA,
            tc.tile_pool(name="psB", bufs=1, space="PSUM") as psB,
            tc.tile_pool(name="psX", bufs=2, space="PSUM") as psX,
        ):
            # ---- DMA issue: xT quarters first on sync (F1B's first chunks
            # unblock after half of xT); small params on idle engine queues;
            # madd split across sync+gpsimd --------------------------------
            xT_sb = const.tile([P, CT, N], F16)
            for nh in range(2):
                for ct in range(CT):
                    nc.sync.dma_start(
                        xT_sb[:, ct, nh * NH:(nh + 1) * NH],
                        xT[ct * P:(ct + 1) * P, nh * NH:(nh + 1) * NH])
            wab_sb = const.tile([P, CT, 2, P], F16)
            for ct in range(CT):
                nc.scalar.dma_start(wab_sb[:, ct], wab[ct * P:(ct + 1) * P])
            w2a_sb = const.tile([P, CT, F2 + 2], F16)
            for ct in range(CT):
                nc.scalar.dma_start(w2a_sb[:, ct, :],
                                    w2a[ct * P:(ct + 1) * P, :])
            wlT3_sb = const.tile([P, 3, F], F16)
            for c in range(3):
                nc.gpsimd.dma_start(wlT3_sb[:, c], wlT3[c])
            blc_sb = const.tile([F, 1], F32)
            nc.gpsimd.dma_start(blc_sb[:], blc[:])
            madd_sb = [const.tile([P, N], F16, tag=f"madd{j}",
                                  name=f"madd_sb{j}")
                       for j in range(NT)]
            for jt in range(NT):
                eng = nc.sync if jt % 2 == 0 else nc.gpsimd
                eng.dma_start(madd_sb[jt][:], madd[jt * P:(jt + 1) * P, :])

            # ---- phase A -------------------------------------------------
            F1B_sb = [const.tile([P, N], F16, tag=f"f1b{hl}",
                                 name=f"F1B_sb{hl}") for hl in range(2)]
            h2_sb = const.tile([P, NT, F2], F16)  # [n, nt, 2F] both heads
            f2c_sb = const.tile([P, 2, NT], F32)
            sc_sb = const.tile([P, 2, NT], F32)
            sch_sb = const.tile([P, 3, 2], F32)  # split-tile accum halves
            rc_sb = const.tile([P, 2, NT], F32)

            # F1B head 0 via the host-replicated stationary; copies on the
            # scalar engine (they precede all exps in its FIFO). F1B psums
            # live in psX so the h2 stream (psA) doesn't gate their rotation.
            # high_priority keeps the scheduler from statically interleaving
            # the h2 matmul stream ahead of this EXP[0]-critical chain.
            with tc.high_priority():
                for icc in range(NIC):
                    ps = psX.tile([P, IC], F32, tag="psum_x")
                    for ct in range(CT):
                        nc.tensor.matmul(
                            ps[:], wab_sb[:, ct, 0],
                            xT_sb[:, ct, icc * IC:(icc + 1) * IC],
                            start=(ct == 0), stop=(ct == CT - 1))
                    nc.scalar.copy(F1B_sb[0][:, icc * IC:(icc + 1) * IC],
                                   ps[:])

            # h2 + f2 share stationaries (xT chunks); f2 in its own psum,
            # one tile per 4-nt group so grp0's f2 column (the EXP[0] gate)
            # is copyable right after grp0's matmuls. h2/f2 copies for
            # grp >= 1 are deferred into loop slack.
            f2ps = [psX.tile([P, 2, 4], F32, tag="psum_x", name=f"f2ps{i}")
                    for i in range(4)]
            h2grp = []
            for grp in range(4):
                ps = psA.tile([P, 4, F2], F32, tag="psum_a")
                h2grp.append(ps)
                for k in range(4):
                    nt = grp * 4 + k
                    for ct in range(CT):
                        nc.tensor.matmul(
                            ps[:, k, :],
                            xT_sb[:, ct, nt * P:(nt + 1) * P],
                            w2a_sb[:, ct, :F2],
                            start=(ct == 0), stop=(ct == CT - 1))
                        nc.tensor.matmul(
                            f2ps[grp][:, :, k],
                            xT_sb[:, ct, nt * P:(nt + 1) * P],
                            w2a_sb[:, ct, F2:],
                            start=(ct == 0), stop=(ct == CT - 1))
            nc.vector.tensor_copy(h2_sb[:, 0:4, :], h2grp[0][:])
            nc.vector.tensor_copy(f2c_sb[:, :, 0:4], f2ps[0][:])

            # ---- hot loop (+ deferred off-critical work in loop slack) ---
            hpT2 = psB.tile([P, N], F32, tag="hpT")
            catT_own = const.tile([P, NH], F16)
            catS = [const.tile([P, NH], F16, tag=f"catS{s}",
                               name=f"catS{s}") for s in range(2)]
            send_sb = const.tile([P, NH], F16)

            def deferred(hl, jt):
                if hl == 0 and 1 <= jt <= 3:
                    # h2 + f2 psum copies for grp 1..3 (deferred off the
                    # EXP[0]-critical vector queue prefix)
                    grp = jt
                    nc.vector.tensor_copy(
                        f2c_sb[:, :, grp * 4:(grp + 1) * 4], f2ps[grp][:])
                    nc.vector.tensor_copy(
                        h2_sb[:, grp * 4:(grp + 1) * 4, :], h2grp[grp][:])
                if hl == 0 and 6 <= jt < 6 + NIC:
                    # F1B head 1, one i-chunk per slot (vector copies)
                    icc = jt - 6
                    ps = psX.tile([P, IC], F32, tag="psum_x")
                    for ct in range(CT):
                        nc.tensor.matmul(
                            ps[:], wab_sb[:, ct, 1],
                            xT_sb[:, ct, icc * IC:(icc + 1) * IC],
                            start=(ct == 0), stop=(ct == CT - 1))
                    nc.vector.tensor_copy(
                        F1B_sb[1][:, icc * IC:(icc + 1) * IC], ps[:])
                if hl == 1 and jt == 7:
                    nc.sync.dma_start(catS[0][0:F, :], cc_out[0][0])
                    nc.sync.dma_start(catS[1][0:F, :], cc_out[0][1])

            for hl in range(2):
                prev = None  # software pipeline: custom(jt+1) before norm(jt)
                for jt in range(NT):
                    lk = vm_pool.tile([P, N], F16, tag="lk")
                    em = em_pool.tile([P, N], F16, tag="em")
                    if hl == 0 and jt < 3:
                        # split the first tiles into i-halves so the exp
                        # stream starts on partial F1B (its first chunks
                        # land ~7us before the last); accum halves summed
                        for h in range(2):
                            sl = slice(h * NH, (h + 1) * NH)
                            bi = nc.vector._custom_dve(
                                LEAKY_PM2X_ANT, out=lk[:, sl],
                                in0=F1B_sb[hl][:, sl],
                                in1=madd_sb[jt][:, sl],
                                s0=f2c_sb[:, hl, jt:jt + 1],
                                s1=float(ALPHA))
                            bi.ins.perf_max = 1
                            nc.scalar.activation(
                                em[:, sl], lk[:, sl], EXPF,
                                accum_out=sch_sb[:, jt, h:h + 1])
                        nc.vector.tensor_tensor(
                            sc_sb[:, hl, jt:jt + 1], sch_sb[:, jt, 0:1],
                            sch_sb[:, jt, 1:2], op=ADD)
                    else:
                        bi = nc.vector._custom_dve(
                            LEAKY_PM2X_ANT, out=lk[:], in0=F1B_sb[hl][:],
                            in1=madd_sb[jt][:], s0=f2c_sb[:, hl, jt:jt + 1],
                            s1=float(ALPHA))
                        bi.ins.perf_max = 1
                        nc.scalar.activation(
                            em[:], lk[:], EXPF,
                            accum_out=sc_sb[:, hl, jt:jt + 1])
                    deferred(hl, jt)
                    if prev is not None:
                        _emit_norm_mm(nc, prev, hl, h2_sb, sc_sb, rc_sb,
                                      g_pool, hpT2)
                    prev = (jt, em)
                # flush: send-half (upper) i-chunks first so their psum
                # banks close and the exchange relu can fire 2 MMs early
                g = _emit_norm_mm(nc, prev, hl, h2_sb, sc_sb, rc_sb,
                                  g_pool, hpT2, iccs=(2, 3))
                nc.vector.tensor_scalar(
                    send_sb[hl * F:(hl + 1) * F, :],
                    hpT2[hl * F:(hl + 1) * F, NH:], 0.0, None, op0=MAX)
                nc.sync.dma_start(cc_in[hl][:],
                                  send_sb[hl * F:(hl + 1) * F, :])
                nc.gpsimd.collective_compute(
                    "AllGather", BYPASS, replica_groups=PAIRS,
                    ins=[cc_in[hl][:]], outs=[cc_out[hl][:]])
                _emit_norm_mm(nc, prev, hl, h2_sb, sc_sb, rc_sb,
                              g_pool, hpT2, iccs=(0, 1), g=g)

            # ---- head-1 epilogue: own-half relus on the idle scalar ------
            nc.scalar.activation(catT_own[0:F, :], hpT2[0:F, :NH],
                                 mybir.ActivationFunctionType.Relu)
            nc.scalar.activation(catT_own[F:F2, :], hpT2[F:F2, :NH],
                                 mybir.ActivationFunctionType.Relu)

            # ---- phase C: final linear over own node half, transposed ----
            # out[f, i] = sum_hf wlT[hf, f] cat[hf, i]: stationary = wl
            # chunks (one LDW each), moving = cat tiles. Own-cat matmuls
            # overlap the gather; two psum banks (i halves).
            psD = [psX.tile([F, IC], F32, tag="psum_x", name=f"psD{i}")
                   for i in range(2)]
            for ih in range(2):
                nc.tensor.matmul(
                    psD[ih][:], wlT3_sb[:, 0],
                    catT_own[:, ih * IC:(ih + 1) * IC],
                    start=True, stop=False)
            nc.sync.dma_start(catS[0][F:F2, :], cc_out[1][0])
            nc.sync.dma_start(catS[1][F:F2, :], cc_out[1][1])
            for ih in range(2):
                for s in range(2):
                    nc.tensor.matmul(
                        psD[ih][:], wlT3_sb[:, 1 + s],
                        catS[s][:, ih * IC:(ih + 1) * IC],
                        start=False, stop=(s == 1))
            # bias (per-partition, fused into the psum drain on scalar),
            # then leaky on vector; out stays [F, NH] (host transposes)
            y_sb = const.tile([F, 2, IC], F32)
            for ih in range(2):
                nc.scalar.activation(
                    y_sb[:, ih, :], psD[ih][:],
                    mybir.ActivationFunctionType.Identity, bias=blc_sb[:])
            yo_sb = const.tile([F, 2, IC], F32)
            nc.vector.scalar_tensor_tensor(
                yo_sb[:], y_sb[:], float(ALPHA), y_sb[:], op0=MULT, op1=MAX)
            nc.sync.dma_start(out.rearrange("f (h i) -> f h i", h=2),
                              yo_sb[:])

    nc.compile()
    return nc


def _emit_norm_mm(nc, prev, hl, h2_sb, sc_sb, rc_sb, g_pool, hpT2,
                  iccs=None, g=None):
    """Normalization + attention matmuls for a finished (jt, em) stage."""
    jt, em = prev
    if g is None:
        nc.vector.reciprocal(rc_sb[:, hl, jt:jt + 1],
                             sc_sb[:, hl, jt:jt + 1])
        g = g_pool.tile([P, F], F16, tag="g")
        nc.vector.tensor_scalar_mul(g[:], h2_sb[:, jt, hl * F:(hl + 1) * F],
                                    rc_sb[:, hl, jt:jt + 1])
    for icc in (range(NIC) if iccs is None else iccs):
        nc.tensor.matmul(
            hpT2[hl * F:(hl + 1) * F, icc * IC:(icc + 1) * IC],
            g[:], em[:, icc * IC:(icc + 1) * IC],
            start=(jt == 0), stop=(jt == NT - 1))
    return g


def get_program():
    if "nc" not in _CACHE:
        _CACHE["nc"] = _build_program()
    return _CACHE["nc"]


def make_in_maps(x, adj, W, a1, a2, Wl, bl):
    x = np.asarray(x, dtype=np.float32)
    adj = np.asarray(adj)
    W = np.asarray(W, dtype=np.float32)
    a1 = np.asarray(a1, dtype=np.float32)
    a2 = np.asarray(a2, dtype=np.float32)
    Wl = np.asarray(Wl, dtype=np.float32)
    bl = np.asarray(bl, dtype=np.float32)

    maddT = (MASKV * adj.T.astype(np.float32)) - MASKV  # [j, i] additive
    madds = [
        np.ascontiguousarray(maddT).astype(np.float16),
        # hp=1: node axis block-swapped on both j and i
        np.ascontiguousarray(
            np.roll(np.roll(maddT, NH, axis=0), NH, axis=1)
        ).astype(np.float16),
    ]
    WlT = np.ascontiguousarray(Wl.T)  # [H*F, F]
    zero_blk = np.zeros((P, F), np.float32)

    in_maps = []
    for k in range(NCORES):
        b, hp = k // 2, k % 2
        w2a = np.concatenate([
            W[2 * hp], W[2 * hp + 1],
            (W[2 * hp] @ a2[2 * hp])[:, None],
            (W[2 * hp + 1] @ a2[2 * hp + 1])[:, None],
        ], axis=1).astype(np.float16)  # [C, 130]
        wab = np.stack([
            np.tile((W[2 * hp] @ a1[2 * hp])[:, None], (1, P)),
            np.tile((W[2 * hp + 1] @ a1[2 * hp + 1])[:, None], (1, P)),
        ], axis=1).astype(np.float16)  # [C, 2, P]
        own = WlT[hp * P:(hp + 1) * P]
        slot0 = zero_blk if hp == 0 else WlT[0:P]       # rank-0 heads (0, 1)
        slot1 = WlT[P:2 * P] if hp == 0 else zero_blk   # rank-1 heads (2, 3)
        wlT3 = np.stack([own, slot0, slot1], axis=0).astype(np.float16)
        xb = x[b].T  # [C, N]
        if hp == 1:
            xb = np.roll(xb, NH, axis=1)
        in_maps.append({
            "xT": np.ascontiguousarray(xb).astype(np.float16),
            "madd": madds[hp],
            "w2a": np.ascontiguousarray(w2a),
            "wab": np.ascontiguousarray(wab),
            "wlT3": np.ascontiguousarray(wlT3),
            "blc": np.ascontiguousarray(bl[:, None]).astype(np.float32),
        })
    return in_maps


def kernel(x, adj, W, a1, a2, Wl, bl, _results=None, **run_kwargs):
    nc = get_program()
    in_maps = make_in_maps(x, adj, W, a1, a2, Wl, bl)
    res = run_bass_kernel_spmd(nc, in_maps, core_ids=list(range(NCORES)),
                               **run_kwargs)
    if _results is not None:
        _results.append(res)
    out = np.empty((B, N, F), dtype=np.float32)
    for b in range(B):
        out[b, :NH] = res.results[2 * b]["out"].T
        out[b, NH:] = res.results[2 * b + 1]["out"].T
    return out


# revision 37
# speedup vs baseline: 1.1221x; 1.0534x over previous
"""GAT layer kernel for Trainium2, 8 NeuronCores.

Sharding: 16 (b, h) pairs -> 8 cores. Core k handles batch b = k//2 and the
head pair hp = k%2 (heads 2*hp, 2*hp+1); each core runs the full N^2
attention for its two heads. The head-mixing linear is split over nodes: the
pair of cores for one batch exchanges relu'd per-head outputs (cat halves)
with two small fp16 AllGathers (head 0's hides behind head 1's hot loop),
then each core computes the final linear + leaky for its own half of the
nodes only. No output AllReduce.

The program is identical on every core (SPMD): hp=1 cores get their node
axis block-swapped on the host (xT columns and madd rows+cols), so "own
half" is always local columns [0, N/2). Which AllGather slot holds the
peer's data is absorbed into the weights: the final linear contracts over
[own-cat, slot0-cat, slot1-cat] with the slot holding this core's own
sent-back data zeroed in wlT3.

Math per (b, h), with softmax over the *i* axis (rows) of e[i, j]:
  h    = x[b] @ W[h]                         [N, F]
  f1_i = h_i . a1,  f2_j = h_j . a2
  v[j, i]  = f1_i + f2_j + M[j, i]           (M = 0 on edge, -150 masked)
  L        = max(v, 0.2*v)                   (= leaky, exp-monotone safe)
  Em[j, i] = exp(L)    ;  s_j = sum_i Em[j, i]   (ACT accum_out, fused)
  g[j, :]  = h[j, :] / s_j
  hpT[f, i] = sum_j g[j, f] * Em[j, i]       (PE, transposed-out layout)
  out[own i] = leaky(relu(hp) cat-4-heads @ Wl.T + bl)

All PE inputs are fp16; the per-head projections are merged into single
[C, 2F] matmuls and f2 rides the same stationaries as h2. F1B (the broadcast
f1 row) comes from a host-replicated stationary (wab), so phase A needs no
ones-outer-product and no f1-row copies. The scalar engine (the pacer) runs
only: 4 F1B copies, then 32x (exp + accum) back to back.
"""

import sys

import numpy as np

sys.path.insert(0, "/opt/trn_rl_repo")

from concourse import bacc, bass, dve_ops, mybir, tile  # noqa: E402
from concourse.bass_utils import run_bass_kernel_spmd  # noqa: E402
from concourse.dve_spec import C0, C1, C2, Spec, Src0, Src1, relu  # noqa: E402

# Fused leaky-relu of a masked outer sum, one DVE pass at 1x:
#   out = leaky(in0 + s0 + in1) = s1*v + imm2*relu(v),  v = in0 + s0 + in1
# (in0 = broadcast f1 row, s0 = per-partition f2, in1 = additive adj mask).
_v = (Src0 + C0) + Src1
LEAKY_MASK_ANT = dve_ops.DveOp(
    "LEAKY_MASK_ANT",
    Spec(
        body=_v * C1 + relu(_v) * C2,
        reference=lambda in0, in1, s0, s1, imm2: (
            lambda v: (v * s1 + np.maximum(v, 0) * imm2).astype(np.float32)
        )(in0.astype(np.float32) + s0 + in1),
    ),
    subdim=False,
    uops_sha={"v3": "61445124be53cf8e", "v4": "fd84e7f03d2c00e0"},
)
if LEAKY_MASK_ANT.name not in dve_ops._SUB_OPCODE_FOR_NAME:
    dve_ops.OPS.append(LEAKY_MASK_ANT)
    dve_ops._SUB_OPCODE_FOR_NAME[LEAKY_MASK_ANT.name] = (
        dve_ops._CUSTOM_DVE_ROW_BASE + len(dve_ops.OPS) - 1)
    dve_ops.CUSTOM_DVE_SPECS[LEAKY_MASK_ANT.name] = LEAKY_MASK_ANT.spec

# Mask-after-leaky variant with a hand-authored 2x_1PORT uop program:
#   out = max(a, a*s1) + in1,  a = in0 + s0
# (identical math given the additive mask rides outside the leaky: exp of
# both is ~0 at masked entries). 4 ALU stages -> lo element in slices 0-3,
# hi element (SRC_*_HI) in slices 4-7, lo result rides the delay chain out
# through WR0_LO=DELAY_0 while the hi result exits via ALU_OUT.
from concourse.dve_spec import lower as _dve_lower, maxx  # noqa: E402
from concourse.dve_uop import (  # noqa: E402
    AluInp, AluOp as UAluOp, DelayInp, DveOpSpec, InpSel, OutPath, OutSel,
)


def _leaky_pm_2x_uop(u1x):
    import copy as _copy
    u = _copy.deepcopy(u1x)
    u.inp = [InpSel.ZERO, InpSel.SRC_0, InpSel.CONST_0, InpSel.CONST_1,
             InpSel.SRC_1, InpSel.SRC_0_HI, InpSel.SRC_1_HI, InpSel.ZERO]
    u.inp_enable = [0, 1, 1, 1, 1, 1, 1, 0]
    u.out = {OutPath.WR0_LO: OutSel.DELAY_0, OutPath.WR0_HI: OutSel.ALU_OUT,
             OutPath.WR1_LO: OutSel.ALU_OUT, OutPath.WR1_HI: OutSel.ALU_OUT}
    u.out_enable = {OutPath.WR0_LO: 1, OutPath.WR0_HI: 1,
                    OutPath.WR1_LO: 0, OutPath.WR1_HI: 0}

    def mk(ref, op, a, b, cap=None):
        dp = _copy.deepcopy(ref)
        dp.op = op
        dp.alu_src0 = a
        dp.alu_src1 = b
        dp.delay = [DelayInp.PREV_DELAY] * 6 + [DelayInp.PREV_ALU_OUT]
        dp.delay_enable = [1, 1, 1, 1, 1, 1, 0]
        dp.alu_out_enable = 1
        if cap is not None:
            dp.delay[cap] = DelayInp.PREV_ALU_OUT
        return dp

    r = u1x.datapath_config[0]
    PA, PD = AluInp.PREV_ALU_OUT, [
        AluInp.PREV_DELAY_0, AluInp.PREV_DELAY_1, AluInp.PREV_DELAY_2,
        AluInp.PREV_DELAY_3, AluInp.PREV_DELAY_4, AluInp.PREV_DELAY_5]
    u.datapath_config = [
        mk(r, UAluOp.ADD, PD[0], PD[1]),            # a_lo = Src0 + C0
        mk(r, UAluOp.MULTIPLY, PA, PD[2], cap=0),   # a_lo*s1; PD0 := a_lo
        mk(r, UAluOp.MAX, PD[0], PA),               # max(a_lo, a_lo*s1)
        mk(r, UAluOp.ADD, PA, PD[3]),               # + Src1 -> lo result
        mk(r, UAluOp.ADD, PD[4], PD[1], cap=0),     # a_hi; PD0 := lo result
        mk(r, UAluOp.MULTIPLY, PA, PD[2], cap=4),   # a_hi*s1; PD4 := a_hi
        mk(r, UAluOp.MAX, PD[4], PA),               # max(a_hi, a_hi*s1)
        mk(r, UAluOp.ADD, PA, PD[5]),               # + Src1_HI -> hi result
    ]
    return u


class _DveOpPerf(dve_ops.DveOp):
    """DveOp whose compiled spec carries a hand-authored 2x_1PORT program."""

    def compile(self, ver):
        key = (self.name, ver)
        if (r := dve_ops._COMPILE_CACHE.get(key)) is not None:
            return r
        uops = _dve_lower(self.spec, ver=ver)
        result = DveOpSpec(
            name=self.name,
            opcode=dve_ops.get_dve_sub_opcode(self.name),
            uops=uops,
            uops_2x=[_leaky_pm_2x_uop(uops[0])],
            perf_max=1,
            rd1_en=True,
        )
        dve_ops._COMPILE_CACHE[key] = result
        return result


_a = Src0 + C0
LEAKY_PM2X_ANT = _DveOpPerf(
    "LEAKY_PM2X_ANT",
    Spec(
        body=maxx(_a, _a * C1) + Src1,
        reference=lambda in0, in1, s0, s1, imm2=0.0: (
            lambda a: (np.maximum(a, a * s1) + in1).astype(np.float32)
        )(in0.astype(np.float32) + s0),
    ),
    subdim=False,
    uops_sha={},
)
if LEAKY_PM2X_ANT.name not in dve_ops._SUB_OPCODE_FOR_NAME:
    dve_ops.OPS.append(LEAKY_PM2X_ANT)
    dve_ops._SUB_OPCODE_FOR_NAME[LEAKY_PM2X_ANT.name] = (
        dve_ops._CUSTOM_DVE_ROW_BASE + len(dve_ops.OPS) - 1)
    dve_ops.CUSTOM_DVE_SPECS[LEAKY_PM2X_ANT.name] = LEAKY_PM2X_ANT.spec

B, N, C, F, H = 4, 2048, 256, 64, 4
P = 128
NT = N // P  # 16 j-tiles / n-chunks
CT = C // P  # 2 contraction tiles over Cin
IC = 512  # i-chunk (matmul moving free dim / psum bank)
NIC = N // IC  # 4
F2 = 2 * F  # merged two-head feature dim (128)
NH = N // 2  # per-core output node half (1024)
NCH = NH // P  # 8 output row chunks
ALPHA = 0.2
MASKV = 150.0  # additive mask magnitude; exp(0.2 * -150) ~ 1e-13
NCORES = 8

F32 = mybir.dt.float32
F16 = mybir.dt.float16
ADD = mybir.AluOpType.add
MULT = mybir.AluOpType.mult
MAX = mybir.AluOpType.max
BYPASS = mybir.AluOpType.bypass
EXPF = mybir.ActivationFunctionType.Exp
PAIRS = [[0, 1], [2, 3], [4, 5], [6, 7]]

_CACHE = {}


def _build_program():
    nc = bacc.Bacc("TRN2", target_bir_lowering=False, debug=False,
                   num_devices=NCORES)

    xT = nc.dram_tensor("xT", [C, N], F16, kind="ExternalInput")
    madd = nc.dram_tensor("madd", [N, N], F16, kind="ExternalInput")
    # w2a[:, 0:128] = per-head W (both local heads); [:, 128:130] = W@a2 cols
    w2a = nc.dram_tensor("w2a", [C, F2 + 2], F16, kind="ExternalInput")
    # wab[:, hl, :] = (W[h] @ a1[h]) column broadcast to 128 -> F1B stationary
    wab = nc.dram_tensor("wab", [C, 2, P], F16, kind="ExternalInput")
    # wlT3[0] = own heads' Wl.T rows; wlT3[1]/[2] = AllGather slot-0/1 heads'
    # rows, with the slot holding this core's own sent-back data zeroed.
    wlT3 = nc.dram_tensor("wlT3", [3, P, F], F16, kind="ExternalInput")
    blc = nc.dram_tensor("blc", [F, 1], F32, kind="ExternalInput")
    # transposed output [F, NH]; the host transposes while gathering
    out = nc.dram_tensor("out", [F, NH], F32, kind="ExternalOutput")

    cc_in = [nc.dram_tensor(f"cc_in{hl}", [F, NH], F16) for hl in range(2)]
    cc_out = [nc.dram_tensor(f"cc_out{hl}", [2, F, NH], F16)
              for hl in range(2)]

    with tile.TileContext(nc) as tc:
        with (
            tc.tile_pool(name="const", bufs=1) as const,
            tc.tile_pool(name="vm", bufs=3) as vm_pool,
            tc.tile_pool(name="em", bufs=3) as em_pool,
            tc.tile_pool(name="g", bufs=4) as g_pool,
            tc.tile_pool(name="psA", bufs=2, space="PSUM") as psA,
            tc.tile_pool(name="psB", bufs=1, space="PSUM") as psB,
            tc.tile_pool(name="psX", bufs=2, space="PSUM") as psX,
        ):
            # ---- DMA issue: xT quarters first on sync (F1B's first chunks
            # unblock after half of xT); small params on idle engine queues;
            # madd split across sync+gpsimd --------------------------------
            xT_sb = const.tile([P, CT, N], F16)
            for nh in range(2):
                for ct in range(CT):
                    nc.sync.dma_start(
                        xT_sb[:, ct, nh * NH:(nh + 1) * NH],
                        xT[ct * P:(ct + 1) * P, nh * NH:(nh + 1) * NH])
            wab_sb = const.tile([P, CT, 2, P], F16)
            for ct in range(CT):
                nc.scalar.dma_start(wab_sb[:, ct], wab[ct * P:(ct + 1) * P])
            w2a_sb = const.tile([P, CT, F2 + 2], F16)
            for ct in range(CT):
                nc.scalar.dma_start(w2a_sb[:, ct, :],
                                    w2a[ct * P:(ct + 1) * P, :])
            wlT3_sb = const.tile([P, 3, F], F16)
            for c in range(3):
                nc.gpsimd.dma_start(wlT3_sb[:, c], wlT3[c])
            blc_sb = const.tile([F, 1], F32)
            nc.gpsimd.dma_start(blc_sb[:], blc[:])
            madd_sb = [const.tile([P, N], F16, tag=f"madd{j}",
                                  name=f"madd_sb{j}")
                       for j in range(NT)]
            for jt in range(NT):
                eng = nc.sync if jt % 2 == 0 else nc.gpsimd
                eng.dma_start(madd_sb[jt][:], madd[jt * P:(jt + 1) * P, :])

            # ---- phase A -------------------------------------------------
            F1B_sb = [const.tile([P, N], F16, tag=f"f1b{hl}",
                                 name=f"F1B_sb{hl}") for hl in range(2)]
            h2_sb = const.tile([P, NT, F2], F16)  # [n, nt, 2F] both heads
            f2c_sb = const.tile([P, 2, NT], F32)
            sc_sb = const.tile([P, 2, NT], F32)
            sch_sb = const.tile([P, 3, 2], F32)  # split-tile accum halves
            rc_sb = const.tile([P, 2, NT], F32)

            # F1B head 0 via the host-replicated stationary; copies on the
            # scalar engine (they precede all exps in its FIFO). F1B psums
            # live in psX so the h2 stream (psA) doesn't gate their rotation.
            # high_priority keeps the scheduler from statically interleaving
            # the h2 matmul stream ahead of this EXP[0]-critical chain.
            with tc.high_priority():
                for icc in range(NIC):
                    ps = psX.tile([P, IC], F32, tag="psum_x")
                    for ct in range(CT):
                        nc.tensor.matmul(
                            ps[:], wab_sb[:, ct, 0],
                            xT_sb[:, ct, icc * IC:(icc + 1) * IC],
                            start=(ct == 0), stop=(ct == CT - 1))
                    nc.scalar.copy(F1B_sb[0][:, icc * IC:(icc + 1) * IC],
                                   ps[:])

            # h2 + f2 share stationaries (xT chunks); f2 in its own psum,
            # one tile per 4-nt group so grp0's f2 column (the EXP[0] gate)
            # is copyable right after grp0's matmuls. h2/f2 copies for
            # grp >= 1 are deferred into loop slack.
            # f2 psums interleave with h2's in psA so f2-grp0 gets a free
            # buffer immediately (psX rotation would chain it behind the
            # late F1B-icc2 copy and stall the first leaky)
            f2ps = []
            h2grp = []
            for grp in range(4):
                ps = psA.tile([P, 4, F2], F32, tag="psum_a")
                h2grp.append(ps)
                fp = psA.tile([P, 2, 4], F32, tag="psum_a",
                              name=f"f2ps{grp}")
                f2ps.append(fp)
                for k in range(4):
                    nt = grp * 4 + k
                    for ct in range(CT):
                        nc.tensor.matmul(
                            ps[:, k, :],
                            xT_sb[:, ct, nt * P:(nt + 1) * P],
                            w2a_sb[:, ct, :F2],
                            start=(ct == 0), stop=(ct == CT - 1))
                        nc.tensor.matmul(
                            f2ps[grp][:, :, k],
                            xT_sb[:, ct, nt * P:(nt + 1) * P],
                            w2a_sb[:, ct, F2:],
                            start=(ct == 0), stop=(ct == CT - 1))
            nc.vector.tensor_copy(h2_sb[:, 0:4, :], h2grp[0][:])
            nc.vector.tensor_copy(f2c_sb[:, :, 0:4], f2ps[0][:])

            # ---- hot loop (+ deferred off-critical work in loop slack) ---
            hpT2 = psB.tile([P, N], F32, tag="hpT")
            catT_own = const.tile([P, NH], F16)
            catS = [const.tile([P, NH], F16, tag=f"catS{s}",
                               name=f"catS{s}") for s in range(2)]
            send_sb = const.tile([P, NH], F16)

            def deferred(hl, jt):
                if hl == 0 and 1 <= jt <= 3:
                    # h2 + f2 psum copies for grp 1..3 (deferred off the
                    # EXP[0]-critical vector queue prefix)
                    grp = jt
                    nc.vector.tensor_copy(
                        f2c_sb[:, :, grp * 4:(grp + 1) * 4], f2ps[grp][:])
                    nc.vector.tensor_copy(
                        h2_sb[:, grp * 4:(grp + 1) * 4, :], h2grp[grp][:])
                if hl == 0 and 6 <= jt < 6 + NIC:
                    # F1B head 1, one i-chunk per slot (vector copies)
                    icc = jt - 6
                    ps = psX.tile([P, IC], F32, tag="psum_x")
                    for ct in range(CT):
                        nc.tensor.matmul(
                            ps[:], wab_sb[:, ct, 1],
                            xT_sb[:, ct, icc * IC:(icc + 1) * IC],
                            start=(ct == 0), stop=(ct == CT - 1))
                    nc.vector.tensor_copy(
                        F1B_sb[1][:, icc * IC:(icc + 1) * IC], ps[:])
                if hl == 1 and jt == 7:
                    nc.sync.dma_start(catS[0][0:F, :], cc_out[0][0])
                    nc.sync.dma_start(catS[1][0:F, :], cc_out[0][1])

            for hl in range(2):
                prev = None  # software pipeline: custom(jt+1) before norm(jt)
                for jt in range(NT):
                    lk = vm_pool.tile([P, N], F16, tag="lk")
                    em = em_pool.tile([P, N], F16, tag="em")
                    if hl == 0 and jt < 3:
                        # split the first tiles into i-halves so the exp
                        # stream starts on partial F1B (its first chunks
                        # land ~7us before the last); accum halves summed
                        for h in range(2):
                            sl = slice(h * NH, (h + 1) * NH)
                            bi = nc.vector._custom_dve(
                                LEAKY_PM2X_ANT, out=lk[:, sl],
                                in0=F1B_sb[hl][:, sl],
                                in1=madd_sb[jt][:, sl],
                                s0=f2c_sb[:, hl, jt:jt + 1],
                                s1=float(ALPHA))
                            bi.ins.perf_max = 1
                            nc.scalar.activation(
                                em[:, sl], lk[:, sl], EXPF,
                                accum_out=sch_sb[:, jt, h:h + 1])
                        nc.vector.tensor_tensor(
                            sc_sb[:, hl, jt:jt + 1], sch_sb[:, jt, 0:1],
                            sch_sb[:, jt, 1:2], op=ADD)
                    else:
                        bi = nc.vector._custom_dve(
                            LEAKY_PM2X_ANT, out=lk[:], in0=F1B_sb[hl][:],
                            in1=madd_sb[jt][:], s0=f2c_sb[:, hl, jt:jt + 1],
                            s1=float(ALPHA))
                        bi.ins.perf_max = 1
                        nc.scalar.activation(
                            em[:], lk[:], EXPF,
                            accum_out=sc_sb[:, hl, jt:jt + 1])
                    deferred(hl, jt)
                    if prev is not None:
                        _emit_norm_mm(nc, prev, hl, h2_sb, sc_sb, rc_sb,
                                      g_pool, hpT2)
                    prev = (jt, em)
                # flush: send-half (upper) i-chunks first so their psum
                # banks close and the exchange relu can fire 2 MMs early
                g = _emit_norm_mm(nc, prev, hl, h2_sb, sc_sb, rc_sb,
                                  g_pool, hpT2, iccs=(2, 3))
                nc.vector.tensor_scalar(
                    send_sb[hl * F:(hl + 1) * F, :],
                    hpT2[hl * F:(hl + 1) * F, NH:], 0.0, None, op0=MAX)
                nc.sync.dma_start(cc_in[hl][:],
                                  send_sb[hl * F:(hl + 1) * F, :])
                nc.gpsimd.collective_compute(
                    "AllGather", BYPASS, replica_groups=PAIRS,
                    ins=[cc_in[hl][:]], outs=[cc_out[hl][:]])
                _emit_norm_mm(nc, prev, hl, h2_sb, sc_sb, rc_sb,
                              g_pool, hpT2, iccs=(0, 1), g=g)

            # ---- head-1 epilogue: own-half relus on the idle scalar ------
            nc.scalar.activation(catT_own[0:F, :], hpT2[0:F, :NH],
                                 mybir.ActivationFunctionType.Relu)
            nc.scalar.activation(catT_own[F:F2, :], hpT2[F:F2, :NH],
                                 mybir.ActivationFunctionType.Relu)

            # ---- phase C: final linear over own node half, transposed ----
            # out[f, i] = sum_hf wlT[hf, f] cat[hf, i]: stationary = wl
            # chunks (one LDW each), moving = cat tiles. Own-cat matmuls
            # overlap the gather; two psum banks (i halves).
            psD = [psX.tile([F, IC], F32, tag="psum_x", name=f"psD{i}")
                   for i in range(2)]
            for ih in range(2):
                nc.tensor.matmul(
                    psD[ih][:], wlT3_sb[:, 0],
                    catT_own[:, ih * IC:(ih + 1) * IC],
                    start=True, stop=False)
            nc.sync.dma_start(catS[0][F:F2, :], cc_out[1][0])
            nc.sync.dma_start(catS[1][F:F2, :], cc_out[1][1])
            for ih in range(2):
                for s in range(2):
                    nc.tensor.matmul(
                        psD[ih][:], wlT3_sb[:, 1 + s],
                        catS[s][:, ih * IC:(ih + 1) * IC],
                        start=False, stop=(s == 1))
            # bias (per-partition, fused into the psum drain on scalar),
            # then leaky on vector; out stays [F, NH] (host transposes)
            y_sb = const.tile([F, 2, IC], F32)
            for ih in range(2):
                nc.scalar.activation(
                    y_sb[:, ih, :], psD[ih][:],
                    mybir.ActivationFunctionType.Identity, bias=blc_sb[:])
            yo_sb = const.tile([F, 2, IC], F32)
            nc.vector.scalar_tensor_tensor(
                yo_sb[:], y_sb[:], float(ALPHA), y_sb[:], op0=MULT, op1=MAX)
            nc.sync.dma_start(out.rearrange("f (h i) -> f h i", h=2),
                              yo_sb[:])

    nc.compile()
    return nc


def _emit_norm_mm(nc, prev, hl, h2_sb, sc_sb, rc_sb, g_pool, hpT2,
                  iccs=None, g=None):
    """Normalization + attention matmuls for a finished (jt, em) stage."""
    jt, em = prev
    if g is None:
        nc.vector.reciprocal(rc_sb[:, hl, jt:jt + 1],
                             sc_sb[:, hl, jt:jt + 1])
        g = g_pool.tile([P, F], F16, tag="g")
        nc.vector.tensor_scalar_mul(g[:], h2_sb[:, jt, hl * F:(hl + 1) * F],
                                    rc_sb[:, hl, jt:jt + 1])
    for icc in (range(NIC) if iccs is None else iccs):
        nc.tensor.matmul(
            hpT2[hl * F:(hl + 1) * F, icc * IC:(icc + 1) * IC],
            g[:], em[:, icc * IC:(icc + 1) * IC],
            start=(jt == 0), stop=(jt == NT - 1))
    return g


def get_program():
    if "nc" not in _CACHE:
        _CACHE["nc"] = _build_program()
    return _CACHE["nc"]


def make_in_maps(x, adj, W, a1, a2, Wl, bl):
    x = np.asarray(x, dtype=np.float32)
    adj = np.asarray(adj)
    W = np.asarray(W, dtype=np.float32)
    a1 = np.asarray(a1, dtype=np.float32)
    a2 = np.asarray(a2, dtype=np.float32)
    Wl = np.asarray(Wl, dtype=np.float32)
    bl = np.asarray(bl, dtype=np.float32)

    maddT = (MASKV * adj.T.astype(np.float32)) - MASKV  # [j, i] additive
    madds = [
        np.ascontiguousarray(maddT).astype(np.float16),
        # hp=1: node axis block-swapped on both j and i
        np.ascontiguousarray(
            np.roll(np.roll(maddT, NH, axis=0), NH, axis=1)
        ).astype(np.float16),
    ]
    WlT = np.ascontiguousarray(Wl.T)  # [H*F, F]
    zero_blk = np.zeros((P, F), np.float32)

    in_maps = []
    for k in range(NCORES):
        b, hp = k // 2, k % 2
        w2a = np.concatenate([
            W[2 * hp], W[2 * hp + 1],
            (W[2 * hp] @ a2[2 * hp])[:, None],
            (W[2 * hp + 1] @ a2[2 * hp + 1])[:, None],
        ], axis=1).astype(np.float16)  # [C, 130]
        wab = np.stack([
            np.tile((W[2 * hp] @ a1[2 * hp])[:, None], (1, P)),
            np.tile((W[2 * hp + 1] @ a1[2 * hp + 1])[:, None], (1, P)),
        ], axis=1).astype(np.float16)  # [C, 2, P]
        own = WlT[hp * P:(hp + 1) * P]
        slot0 = zero_blk if hp == 0 else WlT[0:P]       # rank-0 heads (0, 1)
        slot1 = WlT[P:2 * P] if hp == 0 else zero_blk   # rank-1 heads (2, 3)
        wlT3 = np.stack([own, slot0, slot1], axis=0).astype(np.float16)
        xb = x[b].T  # [C, N]
        if hp == 1:
            xb = np.roll(xb, NH, axis=1)
        in_maps.append({
            "xT": np.ascontiguousarray(xb).astype(np.float16),
            "madd": madds[hp],
            "w2a": np.ascontiguousarray(w2a),
            "wab": np.ascontiguousarray(wab),
            "wlT3": np.ascontiguousarray(wlT3),
            "blc": np.ascontiguousarray(bl[:, None]).astype(np.float32),
        })
    return in_maps


def kernel(x, adj, W, a1, a2, Wl, bl, _results=None, **run_kwargs):
    nc = get_program()
    in_maps = make_in_maps(x, adj, W, a1, a2, Wl, bl)
    res = run_bass_kernel_spmd(nc, in_maps, core_ids=list(range(NCORES)),
                               **run_kwargs)
    if _results is not None:
        _results.append(res)
    out = np.empty((B, N, F), dtype=np.float32)
    for b in range(B):
        out[b, :NH] = res.results[2 * b]["out"].T
        out[b, NH:] = res.results[2 * b + 1]["out"].T
    return out
